# revision 12
# baseline (speedup 1.0000x reference)
"""C2Q attention kernel for 8 TRN2 NeuronCores — ragged-aware.

Math (per batch):
    score  = (o_c @ W @ o_q.T + (o_c @ b) 1^T) / sqrt(H)   [Tc, Tq]
    prob   = softmax_j(score masked at j>=q_len)
    out    = (prob * (i < c_len)) @ o_q                     [Tc, H]

Two exploits make the device program smaller than the dense math:
  * softmax is invariant to a per-row constant, so the bias term
    (o_c@b)1^T cancels exactly -> never computed.
  * by associativity the H x H projection can hit either side:
    (o_c @ W) @ o_q.T  or  o_c @ (W @ o_q.T).  Each slot picks the
    side with the shorter length, so the Linear costs 64*min(q,c)
    PE-rows instead of 64*Tq.

Ragged scheduling: the program is compiled AT RUNTIME for the actual
lengths.  The 32 batches are grouped into 4 slots x 8 cores so that
each slot's compile-time shape (q~, c~) = componentwise max over its 8
batches (grouping chosen by local search to minimize total PE rows).
All cores run the identical 4-slot program on their own batch of each
slot -> SPMD holds, but ~25% of the dense FLOPs are never issued.

Device layout per slot (everything lands K-on-partitions, no on-chip
transposes):
    proj   = 8 psum groups, free dim = min(q~,c~)     [128, 8*L] f16
    e[j,i] = exp(score/32 + qbias[j]) per j-tile      [<=128, c~] f16
             (qbias in {0,-60000} -> masked j gives exactly 0)
    ctx    = e.T @ [1 | o_q] in 3 free-blocks of ~342; the ones column
             makes d[i] = sum_j e[j,i] land in psum col 0, already
             per-partition -> reciprocal feeds the eviction scale.
c_len masking is host-side (only rows < c_len are copied out).
"""

import os
import sys

import numpy as np

if "/opt/trn_rl_repo" not in sys.path:
    sys.path.insert(0, "/opt/trn_rl_repo")

B, Tc, Tq, H = 32, 512, 512, 1024
N_CORES = 8
N_SLOTS = B // N_CORES  # 4
KT = H // 128  # contraction tiles over features (8)
OW = 1032  # oqN slab block width: [ones | h0..h1023] padded
SCALE = 1.0 / 32.0  # 1/sqrt(H)
NEG16 = np.float16(-60000.0)  # exp(x - 60000) == 0 exactly in fp32

CTX_BLOCKS = [(0, 342), (342, 684), (684, 1025)]  # cols of [1 | h...]


def _r16(x: int) -> int:
    return -(-int(x) // 16) * 16


def _rows(qm: int, cm: int) -> int:
    """PE row cost of one slot with shape (q~, c~)."""
    q, c = _r16(qm), _r16(cm)
    jt, it = -(-q // 128), -(-c // 128)
    return 64 * min(q, c) + 8 * jt * c + it * jt * 1025


def _group_batches(q_len, c_len):
    """Partition 32 batches into 4 groups of 8 minimizing slot-max cost."""
    import random

    rng = random.Random(12345)
    n = len(q_len)

    def total(groups):
        return sum(
            _rows(max(q_len[i] for i in g), max(c_len[i] for i in g))
            for g in groups
        )

    best_t, best_g = None, None
    for trial in range(12):
        order = sorted(
            range(n), key=lambda i: -(q_len[i] * 1024 + c_len[i])
        ) if trial == 0 else rng.sample(range(n), n)
        groups = [order[i * 8 : (i + 1) * 8] for i in range(N_SLOTS)]
        cur = total(groups)
        for _ in range(20000):
            g1, g2 = rng.sample(range(N_SLOTS), 2)
            i1, i2 = rng.randrange(8), rng.randrange(8)
            groups[g1][i1], groups[g2][i2] = groups[g2][i2], groups[g1][i1]
            t = total(groups)
            if t <= cur:
                cur = t
            else:
                groups[g1][i1], groups[g2][i2] = groups[g2][i2], groups[g1][i1]
        if best_t is None or cur < best_t:
            best_t, best_g = cur, [list(g) for g in groups]
    # order slots by descending cost: the big slot rides out the DMA ramp
    # (most PE work per input byte), the small slot gives a short tail
    costs = [
        _rows(max(q_len[i] for i in g), max(c_len[i] for i in g))
        for g in best_g
    ]
    order = sorted(range(N_SLOTS), key=lambda s: -costs[s])
    return [best_g[s] for s in order]


def _build_program(slots):
    """slots: list of dicts with qt, ct, jt, it, side ('q'|'c')."""
    import concourse.bacc as bacc
    import concourse.mybir as mybir
    import concourse.tile as tile

    f32 = mybir.dt.float32
    f16 = mybir.dt.float16
    nc = bacc.Bacc("TRN2", debug=False)

    need_q = any(s["side"] == "q" for s in slots)
    need_c = any(s["side"] == "c" for s in slots)

    wtq_d = nc.declare_dram_parameter("wtq", [128, 4, KT, 256], f16, isOutput=False) if need_q else None
    wtc_d = nc.declare_dram_parameter("wtc", [128, 4, KT, 256], f16, isOutput=False) if need_c else None
    oqT_d, ocT_d, oqN_d, out_d = [], [], [], []
    for s, sl in enumerate(slots):
        qt, ct, jt, it = sl["qt"], sl["ct"], sl["jt"], sl["it"]
        oqT_d.append(nc.declare_dram_parameter(f"oqT{s}", [128, KT * qt + jt], f16, isOutput=False))
        ocT_d.append(nc.declare_dram_parameter(f"ocT{s}", [128, KT * ct], f16, isOutput=False))
        oqN_d.append(nc.declare_dram_parameter(f"oqN{s}", [128, jt * OW], f16, isOutput=False))
        out_d.append(nc.declare_dram_parameter(f"out{s}", [ct, H], f16, isOutput=True))

    with tile.TileContext(nc) as tc:
        with (
            tc.tile_pool(name="const", bufs=1) as cpool,
            tc.tile_pool(name="inp", bufs=2) as ipool,
            tc.tile_pool(name="work", bufs=1) as wpool,
            tc.tile_pool(name="outp", bufs=3) as opool,
            tc.tile_pool(name="ps_acc", bufs=2, space="PSUM") as ps_acc,
            tc.tile_pool(name="ps_ctx", bufs=2, space="PSUM") as ps_ctx,
        ):
            wtq = cpool.tile([128, 4, KT, 256], f16, tag="wtq", name="wtq") if need_q else None
            wtc = cpool.tile([128, 4, KT, 256], f16, tag="wtc", name="wtc") if need_c else None

            # per-slot state carried between emission phases
            st = [dict() for _ in slots]

            def warmup():
                """~3us of throwaway matmuls at program start.  The PE's HAM
                clock gate needs ~3.4us of sustained activity to lift the
                default 4/8 throttle; these run during the input-DMA ramp
                (otherwise dead time) so the first real matmul starts at
                2.4GHz instead of 1.2."""
                wsrc = cpool.tile([128, 640], f16, tag="warm", name="warm_src")
                nc.vector.memset(wsrc[:, :], 0.0)
                for i in range(8):
                    wp = ps_acc.tile([128, 512], f32, tag="acc", name=f"warm{i}")
                    nc.tensor.matmul(
                        wp[:, :512], wsrc[:, :128], wsrc[:, 128:640],
                        start=True, stop=True,
                    )

            def dma_inputs(s, fine):
                sl = slots[s]
                qt, ct, jt = sl["qt"], sl["ct"], sl["jt"]
                oqT = ipool.tile([128, KT * qt + jt], f16, tag="oqT", name=f"oqT_{s}")
                ocT = ipool.tile([128, KT * ct], f16, tag="ocT", name=f"ocT_{s}")
                oqN = ipool.tile([128, jt * OW], f16, tag="oqN", name=f"oqN_{s}")
                st[s].update(oqT=oqT, ocT=ocT, oqN=oqN)
                if fine:
                    # ramp schedule: DMA bandwidth scales hard with packet
                    # size (~130GB/s at 1KB rows vs ~420GB/s at 4-8KB rows),
                    # so stream the moving tensor and the first wt o-block in
                    # k-HALF chunks: rows stay >=2KB contiguous while lin0's
                    # first og-pair can start after half 0 lands
                    w_slab = wtq if sl["side"] == "q" else wtc
                    w_d = wtq_d if sl["side"] == "q" else wtc_d
                    L = qt if sl["side"] == "q" else ct
                    mov, mov_d = (oqT, oqT_d[s]) if sl["side"] == "q" else (ocT, ocT_d[s])
                    oth, oth_d = (ocT, ocT_d[s]) if sl["side"] == "q" else (oqT, oqT_d[s])
                    # 4-way k-pair descriptors: a single DMA descriptor moves
                    # ~130-150GB/s (engine-serial) and the SYNC queue issues
                    # descriptors at only ~650ns each, so the weight slab
                    # rides the otherwise-idle SCALAR queue — both queues
                    # issue in parallel and the transfers land on distinct
                    # engines
                    kh = 2
                    for h in range(4):
                        lo, hi = h * kh * L, (h + 1) * kh * L
                        if h == 3 and mov is oqT:
                            hi += jt  # qb bias columns ride with the last chunk
                        nc.sync.dma_start(out=mov[:, lo:hi], in_=mov_d[:, lo:hi])
                        nc.scalar.dma_start(
                            out=w_slab[:, 0:1, h * kh : (h + 1) * kh, :],
                            in_=w_d[:, 0:1, h * kh : (h + 1) * kh, :],
                        )
                    for ob in range(1, 4):
                        nc.scalar.dma_start(out=w_slab[:, ob], in_=w_d[:, ob])
                        if ob == 2:
                            # the score-side slab is needed ~12us in; for
                            # side-c it also carries the qb bias columns
                            nc.sync.dma_start(out=oth, in_=oth_d[:, :])
                    other_w = wtc if (sl["side"] == "q" and need_c) else (wtq if (sl["side"] == "c" and need_q) else None)
                    other_wd = wtc_d if sl["side"] == "q" else wtq_d
                    if other_w is not None:
                        nc.scalar.dma_start(out=other_w[:, :2], in_=other_wd[:, :2])
                        nc.scalar.dma_start(out=other_w[:, 2:], in_=other_wd[:, 2:])
                else:
                    nc.sync.dma_start(out=oqT, in_=oqT_d[s][:, :])
                    nc.sync.dma_start(out=ocT, in_=ocT_d[s][:, :])
                # all DMA stays on the sync queue: waking the GpSimd queue
                # costs the PE its boost p-state (measured 2.37 -> 2.0 GHz)
                nc.sync.dma_start(out=oqN, in_=oqN_d[s][:, :])

            def linear_gen(s):
                """Yield one emission step (matmul / eviction) at a time so
                ctx(s-1) can interleave them into its eviction bubbles."""
                sl = slots[s]
                qt, ct = sl["qt"], sl["ct"]
                L = qt if sl["side"] == "q" else ct
                w_slab = wtq if sl["side"] == "q" else wtc
                mov = st[s]["oqT"] if sl["side"] == "q" else st[s]["ocT"]
                proj = wpool.tile([128, KT * 512], f16, tag="proj", name=f"proj_{s}")
                st[s]["proj"] = proj
                for o in range(KT):
                    ups = ps_acc.tile([128, 512], f32, tag="acc", name=f"ups{o}_{s}")
                    for k in range(KT):
                        nc.tensor.matmul(
                            ups[:, :L],
                            w_slab[:, o // 2, k, (o % 2) * 128 : (o % 2 + 1) * 128],
                            mov[:, k * L : (k + 1) * L],
                            start=(k == 0),
                            stop=(k == KT - 1),
                        )
                        yield
                    nc.vector.tensor_scalar(
                        out=proj[:, o * L : (o + 1) * L],
                        in0=ups[:, :L],
                        scalar1=1.0,
                        scalar2=None,
                        op0=mybir.AluOpType.mult,
                    )

            def linear0():
                """Slot-0 Linear: o-groups 0/1 as a k-interleaved pair (each
                arriving oqT k-chunk feeds two matmuls, matching the DMA
                arrival rate during the ramp); o-groups 2-7 as single groups
                so the ps_acc ring-2 turnaround hides behind the previous
                group's 8 matmuls (pairs grab BOTH ring buffers at once and
                stall ~1us per pair on the eviction chain)."""
                sl = slots[0]
                qt, ct = sl["qt"], sl["ct"]
                L = qt if sl["side"] == "q" else ct
                w_slab = wtq if sl["side"] == "q" else wtc
                mov = st[0]["oqT"] if sl["side"] == "q" else st[0]["ocT"]
                proj = wpool.tile([128, KT * 512], f16, tag="proj", name="proj_0")
                st[0]["proj"] = proj

                def evict(o, up):
                    nc.vector.tensor_scalar(
                        out=proj[:, o * L : (o + 1) * L],
                        in0=up[:, :L],
                        scalar1=1.0,
                        scalar2=None,
                        op0=mybir.AluOpType.mult,
                    )

                ups = [
                    ps_acc.tile([128, 512], f32, tag="acc", name=f"ups{i}_0")
                    for i in range(2)
                ]
                for k in range(KT):
                    for i in range(2):
                        nc.tensor.matmul(
                            ups[i][:, :L],
                            w_slab[:, i // 2, k, (i % 2) * 128 : (i % 2 + 1) * 128],
                            mov[:, k * L : (k + 1) * L],
                            start=(k == 0),
                            stop=(k == KT - 1),
                        )
                        if k == KT - 1:
                            evict(i, ups[i])
                for o in range(2, KT):
                    up = ps_acc.tile([128, 512], f32, tag="acc", name=f"ups{o}_0")
                    for k in range(KT):
                        nc.tensor.matmul(
                            up[:, :L],
                            w_slab[:, o // 2, k, (o % 2) * 128 : (o % 2 + 1) * 128],
                            mov[:, k * L : (k + 1) * L],
                            start=(k == 0),
                            stop=(k == KT - 1),
                        )
                    evict(o, up)

            def drain(gen, n):
                if gen is None:
                    return
                for _ in range(n):
                    if next(gen, StopIteration) is StopIteration:
                        return

            def score(s):
                sl = slots[s]
                qt, ct, jt = sl["qt"], sl["ct"], sl["jt"]
                stat = st[s]["proj"] if sl["side"] == "q" else st[s]["oqT"]
                mov = st[s]["ocT"] if sl["side"] == "q" else st[s]["proj"]
                stat_L = qt  # j-slices always live in qt-wide sections
                mov_L = ct
                qb = st[s]["oqT"][:, KT * qt : KT * qt + jt]
                e_tiles = []
                for t in range(jt):
                    mj = min(128, qt - t * 128)
                    sps = ps_acc.tile([128, 512], f32, tag="acc", name=f"sps{t}_{s}")
                    for o in range(KT):
                        nc.tensor.matmul(
                            sps[:mj, :ct],
                            stat[:, o * stat_L + t * 128 : o * stat_L + t * 128 + mj],
                            mov[:, o * mov_L : (o + 1) * mov_L],
                            start=(o == 0),
                            stop=(o == KT - 1),
                        )
                    e = wpool.tile([128, 512], f16, tag=f"e{t}", name=f"e{t}_{s}")
                    nc.scalar.activation(
                        out=e[:mj, :ct],
                        in_=sps[:mj, :ct],
                        func=mybir.ActivationFunctionType.Exp,
                        bias=qb[:mj, t : t + 1],
                        scale=SCALE,
                    )
                    e_tiles.append(e)
                st[s]["e"] = e_tiles

            def ctx(s, lin):
                """Emit ctx(s); weave next slot's Linear matmuls (lin gen)
                between psum groups so evictions never stall the PE."""
                sl = slots[s]
                qt, ct, jt, it = sl["qt"], sl["ct"], sl["jt"], sl["it"]
                e_tiles, oqN = st[s]["e"], st[s]["oqN"]
                drain(lin, 4)  # cover the last exp's latency
                last_tile = s == N_SLOTS - 1
                for ti in range(it):
                    mi = min(128, ct - ti * 128)
                    fin = last_tile and ti == it - 1
                    r = wpool.tile([128, 1], f32, tag="r", name=f"r{ti}_{s}")
                    osb = opool.tile([128, H], f16, tag="osb", name=f"osb{ti}_{s}")
                    cps = [
                        ps_ctx.tile([128, 342], f32, tag=f"ctx{bi}", name=f"cps{ti}{bi}_{s}")
                        for bi in range(3)
                    ]

                    def mm(bi, t):
                        c0, c1 = CTX_BLOCKS[bi]
                        mj = min(128, qt - t * 128)
                        nc.tensor.matmul(
                            cps[bi][:mi, : c1 - c0],
                            e_tiles[t][:mj, ti * 128 : ti * 128 + mi],
                            oqN[:mj, t * OW + c0 : t * OW + c1],
                            start=(t == 0),
                            stop=(t == jt - 1),
                        )

                    if not fin:
                        # t-outer: the e-slice stationary is shared by the 3
                        # consecutive block matmuls (walrus can reuse the
                        # loaded weights); evictions land during the NEXT
                        # i-tile's stream via the psum ring
                        for t in range(jt):
                            for bi in range(3):
                                mm(bi, t)
                            drain(lin, 5)
                        nc.vector.reciprocal(out=r[:mi], in_=cps[0][:mi, 0:1])
                        nc.scalar.mul(osb[:mi, 0:341], cps[0][:mi, 1:342], r[:mi])
                        nc.vector.tensor_scalar(
                            out=osb[:mi, 341:683],
                            in0=cps[1][:mi, 0:342],
                            scalar1=r[:mi],
                            scalar2=None,
                            op0=mybir.AluOpType.mult,
                        )
                        nc.scalar.mul(osb[:mi, 683:1024], cps[2][:mi, 0:341], r[:mi])
                        nc.sync.dma_start(
                            out=out_d[s][ti * 128 : ti * 128 + mi, :],
                            in_=osb[:mi, :],
                        )
                    else:
                        # final i-tile of the whole program: bi-outer with
                        # evictions pipelined into the matmul stream, then the
                        # out DMA row-split into two full-width descriptors on
                        # two queues (full-H rows keep 2KB packets; parallel
                        # engines halve the last transfer on the critical path)
                        for bi in range(3):
                            for t in range(jt):
                                mm(bi, t)
                            if bi == 0:
                                nc.vector.reciprocal(out=r[:mi], in_=cps[0][:mi, 0:1])
                            elif bi == 1:
                                nc.scalar.mul(
                                    osb[:mi, 0:341], cps[0][:mi, 1:342], r[:mi]
                                )
                                nc.vector.tensor_scalar(
                                    out=osb[:mi, 341:683],
                                    in0=cps[1][:mi, 0:342],
                                    scalar1=r[:mi],
                                    scalar2=None,
                                    op0=mybir.AluOpType.mult,
                                )
                        nc.scalar.mul(osb[:mi, 683:1024], cps[2][:mi, 0:341], r[:mi])
                        h2 = mi // 2
                        nc.sync.dma_start(
                            out=out_d[s][ti * 128 : ti * 128 + h2, :],
                            in_=osb[:h2, :],
                        )
                        nc.gpsimd.dma_start(
                            out=out_d[s][ti * 128 + h2 : ti * 128 + mi, :],
                            in_=osb[h2:mi, :],
                        )

            # PE order: warmup lin0 score0 | ctx0<<lin1 score1 | ctx1<<lin2
            #           score2 | ctx2<<lin3 score3 | ctx3
            warmup()
            dma_inputs(0, fine=True)
            dma_inputs(1, fine=False)
            linear0()
            score(0)
            for s in range(N_SLOTS):
                if s + 2 < N_SLOTS:
                    dma_inputs(s + 2, fine=False)
                lin = linear_gen(s + 1) if s + 1 < N_SLOTS else None
                ctx(s, lin)
                if lin is not None:
                    drain(lin, 1000)  # finish any remaining lin steps
                    score(s + 1)

    nc.compile()
    return nc


def _plan(q_lengths, c_lengths):
    groups = _group_batches(list(map(int, q_lengths)), list(map(int, c_lengths)))
    slots = []
    for g in groups:
        qt = _r16(max(int(q_lengths[i]) for i in g))
        ct = _r16(max(int(c_lengths[i]) for i in g))
        slots.append(
            dict(
                qt=qt, ct=ct,
                jt=-(-qt // 128), it=-(-ct // 128),
                side="q" if qt <= ct else "c",
                batches=list(g),
            )
        )
    return slots


def _host_inputs(o_c, o_q, W, q_lengths, slots):
    """Per-core input maps (host-side sharding + re-layout), all fp16."""
    need_q = any(s["side"] == "q" for s in slots)
    need_c = any(s["side"] == "c" for s in slots)
    maps = [dict() for _ in range(N_CORES)]
    if need_q:
        # wtq[p, ob, k, c] = W[ob*256 + c, k*128 + p]
        wtq = np.ascontiguousarray(
            W.reshape(4, 256, 8, 128).transpose(3, 0, 2, 1)
        ).astype(np.float16)
        for m in maps:
            m["wtq"] = wtq
    if need_c:
        # wtc[p, mb, k, c] = W[k*128 + p, mb*256 + c]
        wtc = np.ascontiguousarray(
            W.reshape(8, 128, 4, 256).transpose(1, 2, 0, 3)
        ).astype(np.float16)
        for m in maps:
            m["wtc"] = wtc

    jidx = np.arange(128)[:, None]  # partition index within a j-tile
    for s, sl in enumerate(slots):
        qt, ct, jt = sl["qt"], sl["ct"], sl["jt"]
        for core, g in enumerate(sl["batches"]):
            oq = o_q[g]  # [Tq, H] f32
            oc = o_c[g]
            # oqT: [p, k*qt + j] = oq[j, k*128+p], + jt qbias columns
            oqT = np.empty((128, KT * qt + jt), np.float16)
            oqT[:, : KT * qt] = (
                oq[:qt].T.reshape(KT, 128, qt).transpose(1, 0, 2).reshape(128, KT * qt)
            )
            ql = int(q_lengths[g])
            tcol = np.arange(jt)[None, :] * 128 + jidx  # [128, jt]
            oqT[:, KT * qt :] = np.where(tcol < ql, np.float16(0.0), NEG16)
            # ocT: [p, k*ct + i] = oc[i, k*128+p]
            ocT = (
                oc[:ct].T.reshape(KT, 128, ct).transpose(1, 0, 2)
                .reshape(128, KT * ct)
            ).astype(np.float16)
            # oqN: per j-tile block [ones | oq rows]
            oqN = np.zeros((128, jt * OW), np.float16)
            for t in range(jt):
                oqN[:, t * OW] = 1.0
                oqN[:, t * OW + 1 : t * OW + 1 + H] = oq[t * 128 : (t + 1) * 128]
            maps[core][f"oqT{s}"] = np.ascontiguousarray(oqT)
            maps[core][f"ocT{s}"] = np.ascontiguousarray(ocT)
            maps[core][f"oqN{s}"] = np.ascontiguousarray(oqN)
    return maps


def kernel(**inputs) -> np.ndarray:
    o_c = np.asarray(inputs["o_c"], dtype=np.float32)
    o_q = np.asarray(inputs["o_q"], dtype=np.float32)
    W = np.asarray(inputs["W"], dtype=np.float32)
    q_lengths = np.asarray(inputs["q_lengths"]).astype(np.int64)
    c_lengths = np.asarray(inputs["c_lengths"]).astype(np.int64)
    # bias is mathematically irrelevant: it adds (o_c@b) per i-row before
    # softmax over j, which softmax cancels exactly.

    from concourse.bass_utils import run_bass_kernel_spmd

    slots = _plan(q_lengths, c_lengths)
    in_maps = _host_inputs(o_c, o_q, W, q_lengths, slots)
    nc = _build_program(slots)

    trace = bool(int(os.environ.get("KERNEL_TRACE", "0")))
    res = run_bass_kernel_spmd(
        nc, in_maps, core_ids=list(range(N_CORES)), trace=trace
    )
    if trace:
        kernel.last_results = res

    out = np.zeros((B, Tc, H), dtype=np.float32)
    for s, sl in enumerate(slots):
        for core, g in enumerate(sl["batches"]):
            cl = int(c_lengths[g])
            out[g, :cl] = res.results[core][f"out{s}"][:cl].astype(np.float32)
    return out



# revision 13
# speedup vs baseline: 1.0110x; 1.0110x over previous
"""C2Q attention kernel for 8 TRN2 NeuronCores — ragged-aware.

Math (per batch):
    score  = (o_c @ W @ o_q.T + (o_c @ b) 1^T) / sqrt(H)   [Tc, Tq]
    prob   = softmax_j(score masked at j>=q_len)
    out    = (prob * (i < c_len)) @ o_q                     [Tc, H]

Two exploits make the device program smaller than the dense math:
  * softmax is invariant to a per-row constant, so the bias term
    (o_c@b)1^T cancels exactly -> never computed.
  * by associativity the H x H projection can hit either side:
    (o_c @ W) @ o_q.T  or  o_c @ (W @ o_q.T).  Each slot picks the
    side with the shorter length, so the Linear costs 64*min(q,c)
    PE-rows instead of 64*Tq.

Ragged scheduling: the program is compiled AT RUNTIME for the actual
lengths.  The 32 batches are grouped into 4 slots x 8 cores so that
each slot's compile-time shape (q~, c~) = componentwise max over its 8
batches (grouping chosen by local search to minimize total PE rows).
All cores run the identical 4-slot program on their own batch of each
slot -> SPMD holds, but ~25% of the dense FLOPs are never issued.

Device layout per slot (everything lands K-on-partitions, no on-chip
transposes):
    proj   = 8 psum groups, free dim = min(q~,c~)     [128, 8*L] f16
    e[j,i] = exp(score/32 + qbias[j]) per j-tile      [<=128, c~] f16
             (qbias in {0,-60000} -> masked j gives exactly 0)
    ctx    = e.T @ [1 | o_q] in 3 free-blocks of ~342; the ones column
             makes d[i] = sum_j e[j,i] land in psum col 0, already
             per-partition -> reciprocal feeds the eviction scale.
c_len masking is host-side (only rows < c_len are copied out).
"""

import os
import sys

import numpy as np

if "/opt/trn_rl_repo" not in sys.path:
    sys.path.insert(0, "/opt/trn_rl_repo")

B, Tc, Tq, H = 32, 512, 512, 1024
N_CORES = 8
N_SLOTS = B // N_CORES  # 4
KT = H // 128  # contraction tiles over features (8)
OW = 1032  # oqN slab block width: [ones | h0..h1023] padded
SCALE = 1.0 / 32.0  # 1/sqrt(H)
NEG16 = np.float16(-60000.0)  # exp(x - 60000) == 0 exactly in fp32

CTX_BLOCKS = [(0, 342), (342, 684), (684, 1025)]  # cols of [1 | h...]


def _r16(x: int) -> int:
    return -(-int(x) // 16) * 16


def _rows(qm: int, cm: int) -> int:
    """PE row cost of one slot with shape (q~, c~)."""
    q, c = _r16(qm), _r16(cm)
    jt, it = -(-q // 128), -(-c // 128)
    return 64 * min(q, c) + 8 * jt * c + it * jt * 1025


def _group_batches(q_len, c_len):
    """Partition 32 batches into 4 groups of 8 minimizing slot-max cost."""
    import random

    rng = random.Random(12345)
    n = len(q_len)

    def total(groups):
        return sum(
            _rows(max(q_len[i] for i in g), max(c_len[i] for i in g))
            for g in groups
        )

    best_t, best_g = None, None
    for trial in range(12):
        order = sorted(
            range(n), key=lambda i: -(q_len[i] * 1024 + c_len[i])
        ) if trial == 0 else rng.sample(range(n), n)
        groups = [order[i * 8 : (i + 1) * 8] for i in range(N_SLOTS)]
        cur = total(groups)
        for _ in range(20000):
            g1, g2 = rng.sample(range(N_SLOTS), 2)
            i1, i2 = rng.randrange(8), rng.randrange(8)
            groups[g1][i1], groups[g2][i2] = groups[g2][i2], groups[g1][i1]
            t = total(groups)
            if t <= cur:
                cur = t
            else:
                groups[g1][i1], groups[g2][i2] = groups[g2][i2], groups[g1][i1]
        if best_t is None or cur < best_t:
            best_t, best_g = cur, [list(g) for g in groups]
    # order slots by descending cost: the big slot rides out the DMA ramp
    # (most PE work per input byte), the small slot gives a short tail
    costs = [
        _rows(max(q_len[i] for i in g), max(c_len[i] for i in g))
        for g in best_g
    ]
    order = sorted(range(N_SLOTS), key=lambda s: -costs[s])
    return [best_g[s] for s in order]


def _build_program(slots):
    """slots: list of dicts with qt, ct, jt, it, side ('q'|'c')."""
    import concourse.bacc as bacc
    import concourse.mybir as mybir
    import concourse.tile as tile

    f32 = mybir.dt.float32
    f16 = mybir.dt.float16
    nc = bacc.Bacc("TRN2", debug=False)

    need_q = any(s["side"] == "q" for s in slots)
    need_c = any(s["side"] == "c" for s in slots)

    wtq_d = nc.declare_dram_parameter("wtq", [128, 4, KT, 256], f16, isOutput=False) if need_q else None
    wtc_d = nc.declare_dram_parameter("wtc", [128, 4, KT, 256], f16, isOutput=False) if need_c else None
    oqT_d, ocT_d, oqN_d, out_d = [], [], [], []
    for s, sl in enumerate(slots):
        qt, ct, jt, it = sl["qt"], sl["ct"], sl["jt"], sl["it"]
        oqT_d.append(nc.declare_dram_parameter(f"oqT{s}", [128, KT * qt + jt], f16, isOutput=False))
        ocT_d.append(nc.declare_dram_parameter(f"ocT{s}", [128, KT * ct], f16, isOutput=False))
        oqN_d.append(nc.declare_dram_parameter(f"oqN{s}", [128, jt * OW], f16, isOutput=False))
        out_d.append(nc.declare_dram_parameter(f"out{s}", [ct, H], f16, isOutput=True))

    with tile.TileContext(nc) as tc:
        with (
            tc.tile_pool(name="const", bufs=1) as cpool,
            tc.tile_pool(name="inp", bufs=2) as ipool,
            tc.tile_pool(name="work", bufs=1) as wpool,
            tc.tile_pool(name="outp", bufs=3) as opool,
            tc.tile_pool(name="ps_acc", bufs=2, space="PSUM") as ps_acc,
            tc.tile_pool(name="ps_ctx", bufs=2, space="PSUM") as ps_ctx,
        ):
            wtq = cpool.tile([128, 4, KT, 256], f16, tag="wtq", name="wtq") if need_q else None
            wtc = cpool.tile([128, 4, KT, 256], f16, tag="wtc", name="wtc") if need_c else None

            # per-slot state carried between emission phases
            st = [dict() for _ in slots]

            def warmup():
                """~3us of throwaway matmuls at program start.  The PE's HAM
                clock gate needs ~3.4us of sustained activity to lift the
                default 4/8 throttle; these run during the input-DMA ramp
                (otherwise dead time) so the first real matmul starts at
                2.4GHz instead of 1.2."""
                wsrc = cpool.tile([128, 640], f16, tag="warm", name="warm_src")
                nc.vector.memset(wsrc[:, :], 0.0)
                for i in range(8):
                    wp = ps_acc.tile([128, 512], f32, tag="acc", name=f"warm{i}")
                    nc.tensor.matmul(
                        wp[:, :512], wsrc[:, :128], wsrc[:, 128:640],
                        start=True, stop=True,
                    )

            def dma_inputs(s, fine):
                sl = slots[s]
                qt, ct, jt = sl["qt"], sl["ct"], sl["jt"]
                oqT = ipool.tile([128, KT * qt + jt], f16, tag="oqT", name=f"oqT_{s}")
                ocT = ipool.tile([128, KT * ct], f16, tag="ocT", name=f"ocT_{s}")
                oqN = ipool.tile([128, jt * OW], f16, tag="oqN", name=f"oqN_{s}")
                st[s].update(oqT=oqT, ocT=ocT, oqN=oqN)
                if fine:
                    # ramp schedule: DMA bandwidth scales hard with packet
                    # size (~130GB/s at 1KB rows vs ~420GB/s at 4-8KB rows),
                    # so stream the moving tensor and the first wt o-block in
                    # k-HALF chunks: rows stay >=2KB contiguous while lin0's
                    # first og-pair can start after half 0 lands
                    w_slab = wtq if sl["side"] == "q" else wtc
                    w_d = wtq_d if sl["side"] == "q" else wtc_d
                    L = qt if sl["side"] == "q" else ct
                    mov, mov_d = (oqT, oqT_d[s]) if sl["side"] == "q" else (ocT, ocT_d[s])
                    oth, oth_d = (ocT, ocT_d[s]) if sl["side"] == "q" else (oqT, oqT_d[s])
                    # 4-way k-pair descriptors: a single DMA descriptor moves
                    # ~130-150GB/s regardless of size (engine-serial), so
                    # parallel descriptors on distinct engines are what buys
                    # aggregate bandwidth during the ramp
                    kh = 2
                    for h in range(4):
                        lo, hi = h * kh * L, (h + 1) * kh * L
                        if h == 3 and mov is oqT:
                            hi += jt  # qb bias columns ride with the last chunk
                        nc.sync.dma_start(out=mov[:, lo:hi], in_=mov_d[:, lo:hi])
                        nc.sync.dma_start(
                            out=w_slab[:, 0:1, h * kh : (h + 1) * kh, :],
                            in_=w_d[:, 0:1, h * kh : (h + 1) * kh, :],
                        )
                    for ob in range(1, 4):
                        nc.sync.dma_start(out=w_slab[:, ob], in_=w_d[:, ob])
                        if ob == 2:
                            # the score-side slab is needed ~12us in; for
                            # side-c it also carries the qb bias columns
                            nc.sync.dma_start(out=oth, in_=oth_d[:, :])
                    other_w = wtc if (sl["side"] == "q" and need_c) else (wtq if (sl["side"] == "c" and need_q) else None)
                    other_wd = wtc_d if sl["side"] == "q" else wtq_d
                    if other_w is not None:
                        nc.sync.dma_start(out=other_w[:, :2], in_=other_wd[:, :2])
                        nc.sync.dma_start(out=other_w[:, 2:], in_=other_wd[:, 2:])
                else:
                    nc.sync.dma_start(out=oqT, in_=oqT_d[s][:, :])
                    nc.sync.dma_start(out=ocT, in_=ocT_d[s][:, :])
                # all DMA stays on the sync queue: waking the GpSimd queue
                # costs the PE its boost p-state (measured 2.37 -> 2.0 GHz)
                nc.sync.dma_start(out=oqN, in_=oqN_d[s][:, :])

            def linear_gen(s):
                """Yield one emission step (matmul / eviction) at a time so
                ctx(s-1) can interleave them into its eviction bubbles."""
                sl = slots[s]
                qt, ct = sl["qt"], sl["ct"]
                L = qt if sl["side"] == "q" else ct
                w_slab = wtq if sl["side"] == "q" else wtc
                mov = st[s]["oqT"] if sl["side"] == "q" else st[s]["ocT"]
                proj = wpool.tile([128, KT * 512], f16, tag="proj", name=f"proj_{s}")
                st[s]["proj"] = proj
                for o in range(KT):
                    ups = ps_acc.tile([128, 512], f32, tag="acc", name=f"ups{o}_{s}")
                    for k in range(KT):
                        nc.tensor.matmul(
                            ups[:, :L],
                            w_slab[:, o // 2, k, (o % 2) * 128 : (o % 2 + 1) * 128],
                            mov[:, k * L : (k + 1) * L],
                            start=(k == 0),
                            stop=(k == KT - 1),
                        )
                        yield
                    nc.vector.tensor_scalar(
                        out=proj[:, o * L : (o + 1) * L],
                        in0=ups[:, :L],
                        scalar1=1.0,
                        scalar2=None,
                        op0=mybir.AluOpType.mult,
                    )

            def linear0():
                """Slot-0 Linear: o-groups 0/1 as a k-interleaved pair (each
                arriving oqT k-chunk feeds two matmuls, matching the DMA
                arrival rate during the ramp); o-groups 2-7 as single groups
                so the ps_acc ring-2 turnaround hides behind the previous
                group's 8 matmuls (pairs grab BOTH ring buffers at once and
                stall ~1us per pair on the eviction chain)."""
                sl = slots[0]
                qt, ct = sl["qt"], sl["ct"]
                L = qt if sl["side"] == "q" else ct
                w_slab = wtq if sl["side"] == "q" else wtc
                mov = st[0]["oqT"] if sl["side"] == "q" else st[0]["ocT"]
                proj = wpool.tile([128, KT * 512], f16, tag="proj", name="proj_0")
                st[0]["proj"] = proj

                def evict(o, up):
                    nc.vector.tensor_scalar(
                        out=proj[:, o * L : (o + 1) * L],
                        in0=up[:, :L],
                        scalar1=1.0,
                        scalar2=None,
                        op0=mybir.AluOpType.mult,
                    )

                ups = [
                    ps_acc.tile([128, 512], f32, tag="acc", name=f"ups{i}_0")
                    for i in range(2)
                ]
                for k in range(KT):
                    for i in range(2):
                        nc.tensor.matmul(
                            ups[i][:, :L],
                            w_slab[:, i // 2, k, (i % 2) * 128 : (i % 2 + 1) * 128],
                            mov[:, k * L : (k + 1) * L],
                            start=(k == 0),
                            stop=(k == KT - 1),
                        )
                        if k == KT - 1:
                            evict(i, ups[i])
                for o in range(2, KT):
                    up = ps_acc.tile([128, 512], f32, tag="acc", name=f"ups{o}_0")
                    for k in range(KT):
                        nc.tensor.matmul(
                            up[:, :L],
                            w_slab[:, o // 2, k, (o % 2) * 128 : (o % 2 + 1) * 128],
                            mov[:, k * L : (k + 1) * L],
                            start=(k == 0),
                            stop=(k == KT - 1),
                        )
                    evict(o, up)

            def drain(gen, n):
                if gen is None:
                    return
                for _ in range(n):
                    if next(gen, StopIteration) is StopIteration:
                        return

            def score(s):
                sl = slots[s]
                qt, ct, jt = sl["qt"], sl["ct"], sl["jt"]
                stat = st[s]["proj"] if sl["side"] == "q" else st[s]["oqT"]
                mov = st[s]["ocT"] if sl["side"] == "q" else st[s]["proj"]
                stat_L = qt  # j-slices always live in qt-wide sections
                mov_L = ct
                qb = st[s]["oqT"][:, KT * qt : KT * qt + jt]
                e_tiles = []
                for t in range(jt):
                    mj = min(128, qt - t * 128)
                    sps = ps_acc.tile([128, 512], f32, tag="acc", name=f"sps{t}_{s}")
                    for o in range(KT):
                        nc.tensor.matmul(
                            sps[:mj, :ct],
                            stat[:, o * stat_L + t * 128 : o * stat_L + t * 128 + mj],
                            mov[:, o * mov_L : (o + 1) * mov_L],
                            start=(o == 0),
                            stop=(o == KT - 1),
                        )
                    e = wpool.tile([128, 512], f16, tag=f"e{t}", name=f"e{t}_{s}")
                    nc.scalar.activation(
                        out=e[:mj, :ct],
                        in_=sps[:mj, :ct],
                        func=mybir.ActivationFunctionType.Exp,
                        bias=qb[:mj, t : t + 1],
                        scale=SCALE,
                    )
                    e_tiles.append(e)
                st[s]["e"] = e_tiles

            def ctx(s, lin):
                """Emit ctx(s); weave next slot's Linear matmuls (lin gen)
                between psum groups so evictions never stall the PE."""
                sl = slots[s]
                qt, ct, jt, it = sl["qt"], sl["ct"], sl["jt"], sl["it"]
                e_tiles, oqN = st[s]["e"], st[s]["oqN"]
                drain(lin, 4)  # cover the last exp's latency
                last_tile = s == N_SLOTS - 1
                for ti in range(it):
                    mi = min(128, ct - ti * 128)
                    fin = last_tile and ti == it - 1
                    r = wpool.tile([128, 1], f32, tag="r", name=f"r{ti}_{s}")
                    osb = opool.tile([128, H], f16, tag="osb", name=f"osb{ti}_{s}")
                    cps = []
                    # evictions are pipelined INTO the block matmul stream:
                    # recip right after block0 (only needs psum col 0), b0/b1
                    # evictions while block2's matmuls stream, split ACT/DVE
                    # so neither engine serializes the i-tile tail.  For the
                    # very last i-tile the out DMA is col-split into 3
                    # descriptors fired per-eviction, taking the final 256KB
                    # transfer (~2us single-descriptor) off the critical path.
                    for bi, (c0, c1) in enumerate(CTX_BLOCKS):
                        cp = ps_ctx.tile([128, 342], f32, tag=f"ctx{bi}", name=f"cps{ti}{bi}_{s}")
                        for t in range(jt):
                            mj = min(128, qt - t * 128)
                            nc.tensor.matmul(
                                cp[:mi, : c1 - c0],
                                e_tiles[t][:mj, ti * 128 : ti * 128 + mi],
                                oqN[:mj, t * OW + c0 : t * OW + c1],
                                start=(t == 0),
                                stop=(t == jt - 1),
                            )
                        cps.append(cp)
                        if bi == 0:
                            nc.vector.reciprocal(out=r[:mi], in_=cp[:mi, 0:1])
                        elif bi == 1:
                            nc.scalar.mul(
                                osb[:mi, 0:341], cps[0][:mi, 1:342], r[:mi]
                            )
                            nc.vector.tensor_scalar(
                                out=osb[:mi, 341:683],
                                in0=cps[1][:mi, 0:342],
                                scalar1=r[:mi],
                                scalar2=None,
                                op0=mybir.AluOpType.mult,
                            )
                        drain(lin, 6)
                    nc.scalar.mul(
                        osb[:mi, 683:1024], cps[2][:mi, 0:341], r[:mi]
                    )
                    if fin:
                        # final transfer of the program: row-split into two
                        # full-width descriptors (2KB rows keep packets big;
                        # the transfers land on two DMA engines in parallel,
                        # halving the last transfer on the critical path)
                        h2 = mi // 2
                        nc.sync.dma_start(
                            out=out_d[s][ti * 128 : ti * 128 + h2, :],
                            in_=osb[:h2, :],
                        )
                        nc.sync.dma_start(
                            out=out_d[s][ti * 128 + h2 : ti * 128 + mi, :],
                            in_=osb[h2:mi, :],
                        )
                    else:
                        nc.sync.dma_start(
                            out=out_d[s][ti * 128 : ti * 128 + mi, :],
                            in_=osb[:mi, :],
                        )

            # PE order: warmup lin0 score0 | ctx0<<lin1 score1 | ctx1<<lin2
            #           score2 | ctx2<<lin3 score3 | ctx3
            warmup()
            dma_inputs(0, fine=True)
            dma_inputs(1, fine=False)
            linear0()
            score(0)
            for s in range(N_SLOTS):
                if s + 2 < N_SLOTS:
                    dma_inputs(s + 2, fine=False)
                lin = linear_gen(s + 1) if s + 1 < N_SLOTS else None
                ctx(s, lin)
                if lin is not None:
                    drain(lin, 1000)  # finish any remaining lin steps
                    score(s + 1)

    nc.compile()
    return nc


def _plan(q_lengths, c_lengths):
    groups = _group_batches(list(map(int, q_lengths)), list(map(int, c_lengths)))
    slots = []
    for g in groups:
        qt = _r16(max(int(q_lengths[i]) for i in g))
        ct = _r16(max(int(c_lengths[i]) for i in g))
        slots.append(
            dict(
                qt=qt, ct=ct,
                jt=-(-qt // 128), it=-(-ct // 128),
                side="q" if qt <= ct else "c",
                batches=list(g),
            )
        )
    return slots


def _host_inputs(o_c, o_q, W, q_lengths, slots):
    """Per-core input maps (host-side sharding + re-layout), all fp16."""
    need_q = any(s["side"] == "q" for s in slots)
    need_c = any(s["side"] == "c" for s in slots)
    maps = [dict() for _ in range(N_CORES)]
    if need_q:
        # wtq[p, ob, k, c] = W[ob*256 + c, k*128 + p]
        wtq = np.ascontiguousarray(
            W.reshape(4, 256, 8, 128).transpose(3, 0, 2, 1)
        ).astype(np.float16)
        for m in maps:
            m["wtq"] = wtq
    if need_c:
        # wtc[p, mb, k, c] = W[k*128 + p, mb*256 + c]
        wtc = np.ascontiguousarray(
            W.reshape(8, 128, 4, 256).transpose(1, 2, 0, 3)
        ).astype(np.float16)
        for m in maps:
            m["wtc"] = wtc

    jidx = np.arange(128)[:, None]  # partition index within a j-tile
    for s, sl in enumerate(slots):
        qt, ct, jt = sl["qt"], sl["ct"], sl["jt"]
        for core, g in enumerate(sl["batches"]):
            oq = o_q[g]  # [Tq, H] f32
            oc = o_c[g]
            # oqT: [p, k*qt + j] = oq[j, k*128+p], + jt qbias columns
            oqT = np.empty((128, KT * qt + jt), np.float16)
            oqT[:, : KT * qt] = (
                oq[:qt].T.reshape(KT, 128, qt).transpose(1, 0, 2).reshape(128, KT * qt)
            )
            ql = int(q_lengths[g])
            tcol = np.arange(jt)[None, :] * 128 + jidx  # [128, jt]
            oqT[:, KT * qt :] = np.where(tcol < ql, np.float16(0.0), NEG16)
            # ocT: [p, k*ct + i] = oc[i, k*128+p]
            ocT = (
                oc[:ct].T.reshape(KT, 128, ct).transpose(1, 0, 2)
                .reshape(128, KT * ct)
            ).astype(np.float16)
            # oqN: per j-tile block [ones | oq rows]
            oqN = np.zeros((128, jt * OW), np.float16)
            for t in range(jt):
                oqN[:, t * OW] = 1.0
                oqN[:, t * OW + 1 : t * OW + 1 + H] = oq[t * 128 : (t + 1) * 128]
            maps[core][f"oqT{s}"] = np.ascontiguousarray(oqT)
            maps[core][f"ocT{s}"] = np.ascontiguousarray(ocT)
            maps[core][f"oqN{s}"] = np.ascontiguousarray(oqN)
    return maps


def kernel(**inputs) -> np.ndarray:
    o_c = np.asarray(inputs["o_c"], dtype=np.float32)
    o_q = np.asarray(inputs["o_q"], dtype=np.float32)
    W = np.asarray(inputs["W"], dtype=np.float32)
    q_lengths = np.asarray(inputs["q_lengths"]).astype(np.int64)
    c_lengths = np.asarray(inputs["c_lengths"]).astype(np.int64)
    # bias is mathematically irrelevant: it adds (o_c@b) per i-row before
    # softmax over j, which softmax cancels exactly.

    from concourse.bass_utils import run_bass_kernel_spmd

    slots = _plan(q_lengths, c_lengths)
    in_maps = _host_inputs(o_c, o_q, W, q_lengths, slots)
    nc = _build_program(slots)

    trace = bool(int(os.environ.get("KERNEL_TRACE", "0")))
    res = run_bass_kernel_spmd(
        nc, in_maps, core_ids=list(range(N_CORES)), trace=trace
    )
    if trace:
        kernel.last_results = res

    out = np.zeros((B, Tc, H), dtype=np.float32)
    for s, sl in enumerate(slots):
        for core, g in enumerate(sl["batches"]):
            cl = int(c_lengths[g])
            out[g, :cl] = res.results[core][f"out{s}"][:cl].astype(np.float32)
    return out



# revision 14
# speedup vs baseline: 1.1780x; 1.1652x over previous
"""C2Q attention kernel for 8 TRN2 NeuronCores — ragged-aware.

Math (per batch):
    score  = (o_c @ W @ o_q.T + (o_c @ b) 1^T) / sqrt(H)   [Tc, Tq]
    prob   = softmax_j(score masked at j>=q_len)
    out    = (prob * (i < c_len)) @ o_q                     [Tc, H]

Two exploits make the device program smaller than the dense math:
  * softmax is invariant to a per-row constant, so the bias term
    (o_c@b)1^T cancels exactly -> never computed.
  * by associativity the H x H projection can hit either side:
    (o_c @ W) @ o_q.T  or  o_c @ (W @ o_q.T).  Each slot picks the
    side with the shorter length, so the Linear costs 64*min(q,c)
    PE-rows instead of 64*Tq.

Ragged scheduling: the program is compiled AT RUNTIME for the actual
lengths.  The 32 batches are grouped into 4 slots x 8 cores so that
each slot's compile-time shape (q~, c~) = componentwise max over its 8
batches (grouping chosen by local search to minimize total PE rows).
All cores run the identical 4-slot program on their own batch of each
slot -> SPMD holds, but ~25% of the dense FLOPs are never issued.

Device layout per slot (everything lands K-on-partitions, no on-chip
transposes):
    proj   = 8 psum groups, free dim = min(q~,c~)     [128, 8*L] f16
    e[j,i] = exp(score/32 + qbias[j]) per j-tile      [<=128, c~] f16
             (qbias in {0,-60000} -> masked j gives exactly 0)
    ctx    = e.T @ [1 | o_q] in 3 free-blocks of ~342; the ones column
             makes d[i] = sum_j e[j,i] land in psum col 0, already
             per-partition -> reciprocal feeds the eviction scale.
c_len masking is host-side (only rows < c_len are copied out).
"""

import os
import sys

import numpy as np

if "/opt/trn_rl_repo" not in sys.path:
    sys.path.insert(0, "/opt/trn_rl_repo")

B, Tc, Tq, H = 32, 512, 512, 1024
N_CORES = 8
N_SLOTS = B // N_CORES  # 4
KT = H // 128  # contraction tiles over features (8)
OW = 1032  # oqN slab block width: [ones | h0..h1023] padded
SCALE = 1.0 / 32.0  # 1/sqrt(H)
NEG16 = np.float16(-60000.0)  # exp(x - 60000) == 0 exactly in fp32

CTX_BLOCKS = [(0, 342), (342, 684), (684, 1025)]  # cols of [1 | h...]


def _r16(x: int) -> int:
    return -(-int(x) // 16) * 16


def _rows(qm: int, cm: int) -> int:
    """PE row cost of one slot with shape (q~, c~)."""
    q, c = _r16(qm), _r16(cm)
    jt, it = -(-q // 128), -(-c // 128)
    return 64 * min(q, c) + 8 * jt * c + it * jt * 1025


def _group_batches(q_len, c_len):
    """Partition 32 batches into 4 groups of 8 minimizing slot-max cost."""
    import random

    rng = random.Random(12345)
    n = len(q_len)

    def total(groups):
        return sum(
            _rows(max(q_len[i] for i in g), max(c_len[i] for i in g))
            for g in groups
        )

    best_t, best_g = None, None
    for trial in range(12):
        order = sorted(
            range(n), key=lambda i: -(q_len[i] * 1024 + c_len[i])
        ) if trial == 0 else rng.sample(range(n), n)
        groups = [order[i * 8 : (i + 1) * 8] for i in range(N_SLOTS)]
        cur = total(groups)
        for _ in range(20000):
            g1, g2 = rng.sample(range(N_SLOTS), 2)
            i1, i2 = rng.randrange(8), rng.randrange(8)
            groups[g1][i1], groups[g2][i2] = groups[g2][i2], groups[g1][i1]
            t = total(groups)
            if t <= cur:
                cur = t
            else:
                groups[g1][i1], groups[g2][i2] = groups[g2][i2], groups[g1][i1]
        if best_t is None or cur < best_t:
            best_t, best_g = cur, [list(g) for g in groups]
    # order slots by descending cost: the big slot rides out the DMA ramp
    # (most PE work per input byte), the small slot gives a short tail
    costs = [
        _rows(max(q_len[i] for i in g), max(c_len[i] for i in g))
        for g in best_g
    ]
    order = sorted(range(N_SLOTS), key=lambda s: -costs[s])
    return [best_g[s] for s in order]


def _build_program(slots):
    """slots: list of dicts with qt, ct, jt, it, side ('q'|'c')."""
    import concourse.bacc as bacc
    import concourse.mybir as mybir
    import concourse.tile as tile

    f32 = mybir.dt.float32
    f16 = mybir.dt.float16
    nc = bacc.Bacc("TRN2", debug=False)

    need_q = any(s["side"] == "q" for s in slots)
    need_c = any(s["side"] == "c" for s in slots)

    wtq_d = nc.declare_dram_parameter("wtq", [128, 4, KT, 256], f16, isOutput=False) if need_q else None
    wtc_d = nc.declare_dram_parameter("wtc", [128, 4, KT, 256], f16, isOutput=False) if need_c else None
    oqT_d, ocT_d, oqN_d, out_d = [], [], [], []
    for s, sl in enumerate(slots):
        qt, ct, jt, it = sl["qt"], sl["ct"], sl["jt"], sl["it"]
        oqT_d.append(nc.declare_dram_parameter(f"oqT{s}", [128, KT * qt + jt], f16, isOutput=False))
        ocT_d.append(nc.declare_dram_parameter(f"ocT{s}", [128, KT * ct], f16, isOutput=False))
        oqN_d.append(nc.declare_dram_parameter(f"oqN{s}", [128, jt * OW], f16, isOutput=False))
        out_d.append(nc.declare_dram_parameter(f"out{s}", [ct, H], f16, isOutput=True))

    with tile.TileContext(nc) as tc:
        with (
            tc.tile_pool(name="const", bufs=1) as cpool,
            tc.tile_pool(name="inp", bufs=2) as ipool,
            tc.tile_pool(name="work", bufs=1) as wpool,
            tc.tile_pool(name="epool", bufs=2) as epool,
            tc.tile_pool(name="outp", bufs=3) as opool,
            tc.tile_pool(name="ps_acc", bufs=2, space="PSUM") as ps_acc,
            tc.tile_pool(name="ps_ctx", bufs=2, space="PSUM") as ps_ctx,
        ):
            wtq = cpool.tile([128, 4, KT, 256], f16, tag="wtq", name="wtq") if need_q else None
            wtc = cpool.tile([128, 4, KT, 256], f16, tag="wtc", name="wtc") if need_c else None

            # per-slot state carried between emission phases
            st = [dict() for _ in slots]

            def warmup():
                """~3us of throwaway matmuls at program start.  The PE's HAM
                clock gate needs ~3.4us of sustained activity to lift the
                default 4/8 throttle; these run during the input-DMA ramp
                (otherwise dead time) so the first real matmul starts at
                2.4GHz instead of 1.2."""
                wsrc = cpool.tile([128, 640], f16, tag="warm", name="warm_src")
                nc.vector.memset(wsrc[:, :], 0.0)
                for i in range(8):
                    wp = ps_acc.tile([128, 512], f32, tag="acc", name=f"warm{i}")
                    nc.tensor.matmul(
                        wp[:, :512], wsrc[:, :128], wsrc[:, 128:640],
                        start=True, stop=True,
                    )

            def dma_inputs(s, fine):
                sl = slots[s]
                qt, ct, jt = sl["qt"], sl["ct"], sl["jt"]
                oqT = ipool.tile([128, KT * qt + jt], f16, tag="oqT", name=f"oqT_{s}")
                ocT = ipool.tile([128, KT * ct], f16, tag="ocT", name=f"ocT_{s}")
                oqN = ipool.tile([128, jt * OW], f16, tag="oqN", name=f"oqN_{s}")
                st[s].update(oqT=oqT, ocT=ocT, oqN=oqN)
                if fine:
                    # ramp schedule: DMA bandwidth scales hard with packet
                    # size (~130GB/s at 1KB rows vs ~420GB/s at 4-8KB rows),
                    # so stream the moving tensor and the first wt o-block in
                    # k-HALF chunks: rows stay >=2KB contiguous while lin0's
                    # first og-pair can start after half 0 lands
                    w_slab = wtq if sl["side"] == "q" else wtc
                    w_d = wtq_d if sl["side"] == "q" else wtc_d
                    L = qt if sl["side"] == "q" else ct
                    mov, mov_d = (oqT, oqT_d[s]) if sl["side"] == "q" else (ocT, ocT_d[s])
                    oth, oth_d = (ocT, ocT_d[s]) if sl["side"] == "q" else (oqT, oqT_d[s])
                    # 4-way k-pair descriptors: a single DMA descriptor moves
                    # ~130-150GB/s regardless of size (engine-serial), so
                    # parallel descriptors on distinct engines are what buys
                    # aggregate bandwidth during the ramp
                    # issue order matters: the sync queue issues one
                    # descriptor per ~650ns, so wt block1 is hoisted between
                    # the k-chunks (og-pair 2 needs it ~9 matmul-groups in,
                    # and a tail-position issue left a ~2us PE gap)
                    kh = 2
                    for h in range(4):
                        lo, hi = h * kh * L, (h + 1) * kh * L
                        if h == 3 and mov is oqT:
                            hi += jt  # qb bias columns ride with the last chunk
                        nc.sync.dma_start(out=mov[:, lo:hi], in_=mov_d[:, lo:hi])
                        nc.sync.dma_start(
                            out=w_slab[:, 0:1, h * kh : (h + 1) * kh, :],
                            in_=w_d[:, 0:1, h * kh : (h + 1) * kh, :],
                        )
                        if h == 1:
                            nc.sync.dma_start(out=w_slab[:, 1], in_=w_d[:, 1])
                    nc.sync.dma_start(out=w_slab[:, 2], in_=w_d[:, 2])
                    # the score-side slab is needed ~12us in; for side-c it
                    # also carries the qb bias columns
                    nc.sync.dma_start(out=oth, in_=oth_d[:, :])
                    nc.sync.dma_start(out=w_slab[:, 3], in_=w_d[:, 3])
                    other_w = wtc if (sl["side"] == "q" and need_c) else (wtq if (sl["side"] == "c" and need_q) else None)
                    other_wd = wtc_d if sl["side"] == "q" else wtq_d
                    if other_w is not None:
                        nc.sync.dma_start(out=other_w[:, :2], in_=other_wd[:, :2])
                        nc.sync.dma_start(out=other_w[:, 2:], in_=other_wd[:, 2:])
                else:
                    nc.sync.dma_start(out=oqT, in_=oqT_d[s][:, :])
                    nc.sync.dma_start(out=ocT, in_=ocT_d[s][:, :])
                # all DMA stays on the sync queue: waking the GpSimd queue
                # costs the PE its boost p-state (measured 2.37 -> 2.0 GHz)
                nc.sync.dma_start(out=oqN, in_=oqN_d[s][:, :])

            def linear_gen(s):
                """Yield one emission step (matmul / eviction) at a time so
                ctx(s-1) can interleave them into its eviction bubbles."""
                sl = slots[s]
                qt, ct = sl["qt"], sl["ct"]
                L = qt if sl["side"] == "q" else ct
                w_slab = wtq if sl["side"] == "q" else wtc
                mov = st[s]["oqT"] if sl["side"] == "q" else st[s]["ocT"]
                proj = wpool.tile([128, KT * 512], f16, tag="proj", name=f"proj_{s}")
                st[s]["proj"] = proj
                for o in range(KT):
                    ups = ps_acc.tile([128, 512], f32, tag="acc", name=f"ups{o}_{s}")
                    for k in range(KT):
                        nc.tensor.matmul(
                            ups[:, :L],
                            w_slab[:, o // 2, k, (o % 2) * 128 : (o % 2 + 1) * 128],
                            mov[:, k * L : (k + 1) * L],
                            start=(k == 0),
                            stop=(k == KT - 1),
                        )
                        yield
                    nc.vector.tensor_scalar(
                        out=proj[:, o * L : (o + 1) * L],
                        in0=ups[:, :L],
                        scalar1=1.0,
                        scalar2=None,
                        op0=mybir.AluOpType.mult,
                    )

            def linear0():
                """Slot-0 Linear: o-groups 0/1 as a k-interleaved pair (each
                arriving oqT k-chunk feeds two matmuls, matching the DMA
                arrival rate during the ramp); o-groups 2-7 as single groups
                so the ps_acc ring-2 turnaround hides behind the previous
                group's 8 matmuls (pairs grab BOTH ring buffers at once and
                stall ~1us per pair on the eviction chain)."""
                sl = slots[0]
                qt, ct = sl["qt"], sl["ct"]
                L = qt if sl["side"] == "q" else ct
                w_slab = wtq if sl["side"] == "q" else wtc
                mov = st[0]["oqT"] if sl["side"] == "q" else st[0]["ocT"]
                proj = wpool.tile([128, KT * 512], f16, tag="proj", name="proj_0")
                st[0]["proj"] = proj

                def evict(o, up):
                    nc.vector.tensor_scalar(
                        out=proj[:, o * L : (o + 1) * L],
                        in0=up[:, :L],
                        scalar1=1.0,
                        scalar2=None,
                        op0=mybir.AluOpType.mult,
                    )

                ups = [
                    ps_acc.tile([128, 512], f32, tag="acc", name=f"ups{i}_0")
                    for i in range(2)
                ]
                for k in range(KT):
                    for i in range(2):
                        nc.tensor.matmul(
                            ups[i][:, :L],
                            w_slab[:, i // 2, k, (i % 2) * 128 : (i % 2 + 1) * 128],
                            mov[:, k * L : (k + 1) * L],
                            start=(k == 0),
                            stop=(k == KT - 1),
                        )
                        if k == KT - 1:
                            evict(i, ups[i])
                for o in range(2, KT):
                    up = ps_acc.tile([128, 512], f32, tag="acc", name=f"ups{o}_0")
                    for k in range(KT):
                        nc.tensor.matmul(
                            up[:, :L],
                            w_slab[:, o // 2, k, (o % 2) * 128 : (o % 2 + 1) * 128],
                            mov[:, k * L : (k + 1) * L],
                            start=(k == 0),
                            stop=(k == KT - 1),
                        )
                    evict(o, up)

            def drain(gen, n):
                if gen is None:
                    return
                for _ in range(n):
                    if next(gen, StopIteration) is StopIteration:
                        return

            def score(s):
                sl = slots[s]
                qt, ct, jt = sl["qt"], sl["ct"], sl["jt"]
                stat = st[s]["proj"] if sl["side"] == "q" else st[s]["oqT"]
                mov = st[s]["ocT"] if sl["side"] == "q" else st[s]["proj"]
                stat_L = qt  # j-slices always live in qt-wide sections
                mov_L = ct
                qb = st[s]["oqT"][:, KT * qt : KT * qt + jt]
                e_tiles = []
                for t in range(jt):
                    mj = min(128, qt - t * 128)
                    sps = ps_acc.tile([128, 512], f32, tag="acc", name=f"sps{t}_{s}")
                    for o in range(KT):
                        nc.tensor.matmul(
                            sps[:mj, :ct],
                            stat[:, o * stat_L + t * 128 : o * stat_L + t * 128 + mj],
                            mov[:, o * mov_L : (o + 1) * mov_L],
                            start=(o == 0),
                            stop=(o == KT - 1),
                        )
                    # ring-2: slot s+1's exp(t) can overwrite while ctx(s)
                    # still reads slot s's e tiles -> no PE bubble at the
                    # score(s+1) handoff
                    e = epool.tile([128, 512], f16, tag=f"e{t}", name=f"e{t}_{s}")
                    nc.scalar.activation(
                        out=e[:mj, :ct],
                        in_=sps[:mj, :ct],
                        func=mybir.ActivationFunctionType.Exp,
                        bias=qb[:mj, t : t + 1],
                        scale=SCALE,
                    )
                    e_tiles.append(e)
                st[s]["e"] = e_tiles

            def ctx(s, lin):
                """Emit ctx(s); weave next slot's Linear matmuls (lin gen)
                between psum groups so evictions never stall the PE."""
                sl = slots[s]
                qt, ct, jt, it = sl["qt"], sl["ct"], sl["jt"], sl["it"]
                e_tiles, oqN = st[s]["e"], st[s]["oqN"]
                drain(lin, 4)  # cover the last exp's latency
                last_tile = s == N_SLOTS - 1
                for ti in range(it):
                    mi = min(128, ct - ti * 128)
                    fin = last_tile and ti == it - 1
                    r = wpool.tile([128, 1], f32, tag="r", name=f"r{ti}_{s}")
                    osb = opool.tile([128, H], f16, tag="osb", name=f"osb{ti}_{s}")
                    cps = []
                    # evictions are pipelined INTO the block matmul stream:
                    # recip right after block0 (only needs psum col 0), b0/b1
                    # evictions while block2's matmuls stream, split ACT/DVE
                    # so neither engine serializes the i-tile tail.
                    for bi, (c0, c1) in enumerate(CTX_BLOCKS):
                        cp = ps_ctx.tile([128, 342], f32, tag=f"ctx{bi}", name=f"cps{ti}{bi}_{s}")
                        for t in range(jt):
                            mj = min(128, qt - t * 128)
                            nc.tensor.matmul(
                                cp[:mi, : c1 - c0],
                                e_tiles[t][:mj, ti * 128 : ti * 128 + mi],
                                oqN[:mj, t * OW + c0 : t * OW + c1],
                                start=(t == 0),
                                stop=(t == jt - 1),
                            )
                        cps.append(cp)
                        if bi == 0:
                            nc.vector.reciprocal(out=r[:mi], in_=cp[:mi, 0:1])
                        elif bi == 1:
                            nc.scalar.mul(
                                osb[:mi, 0:341], cps[0][:mi, 1:342], r[:mi]
                            )
                            nc.vector.tensor_scalar(
                                out=osb[:mi, 341:683],
                                in0=cps[1][:mi, 0:342],
                                scalar1=r[:mi],
                                scalar2=None,
                                op0=mybir.AluOpType.mult,
                            )
                        drain(lin, 6)
                    nc.scalar.mul(
                        osb[:mi, 683:1024], cps[2][:mi, 0:341], r[:mi]
                    )
                    if fin:
                        # final transfer of the program: row-split into two
                        # full-width descriptors (2KB rows keep packets big;
                        # the transfers land on two DMA engines in parallel,
                        # halving the last transfer on the critical path)
                        h2 = mi // 2
                        nc.sync.dma_start(
                            out=out_d[s][ti * 128 : ti * 128 + h2, :],
                            in_=osb[:h2, :],
                        )
                        nc.sync.dma_start(
                            out=out_d[s][ti * 128 + h2 : ti * 128 + mi, :],
                            in_=osb[h2:mi, :],
                        )
                    else:
                        nc.sync.dma_start(
                            out=out_d[s][ti * 128 : ti * 128 + mi, :],
                            in_=osb[:mi, :],
                        )

            # PE order: warmup lin0 score0 | ctx0<<lin1 score1 | ctx1<<lin2
            #           score2 | ctx2<<lin3 score3 | ctx3
            warmup()
            dma_inputs(0, fine=True)
            dma_inputs(1, fine=False)
            linear0()
            score(0)
            for s in range(N_SLOTS):
                if s + 2 < N_SLOTS:
                    dma_inputs(s + 2, fine=False)
                lin = linear_gen(s + 1) if s + 1 < N_SLOTS else None
                ctx(s, lin)
                if lin is not None:
                    drain(lin, 1000)  # finish any remaining lin steps
                    score(s + 1)

    nc.compile()
    return nc


def _plan(q_lengths, c_lengths):
    groups = _group_batches(list(map(int, q_lengths)), list(map(int, c_lengths)))
    slots = []
    for g in groups:
        qt = _r16(max(int(q_lengths[i]) for i in g))
        ct = _r16(max(int(c_lengths[i]) for i in g))
        slots.append(
            dict(
                qt=qt, ct=ct,
                jt=-(-qt // 128), it=-(-ct // 128),
                side="q" if qt <= ct else "c",
                batches=list(g),
            )
        )
    return slots


def _host_inputs(o_c, o_q, W, q_lengths, slots):
    """Per-core input maps (host-side sharding + re-layout), all fp16."""
    need_q = any(s["side"] == "q" for s in slots)
    need_c = any(s["side"] == "c" for s in slots)
    maps = [dict() for _ in range(N_CORES)]
    if need_q:
        # wtq[p, ob, k, c] = W[ob*256 + c, k*128 + p]
        wtq = np.ascontiguousarray(
            W.reshape(4, 256, 8, 128).transpose(3, 0, 2, 1)
        ).astype(np.float16)
        for m in maps:
            m["wtq"] = wtq
    if need_c:
        # wtc[p, mb, k, c] = W[k*128 + p, mb*256 + c]
        wtc = np.ascontiguousarray(
            W.reshape(8, 128, 4, 256).transpose(1, 2, 0, 3)
        ).astype(np.float16)
        for m in maps:
            m["wtc"] = wtc

    jidx = np.arange(128)[:, None]  # partition index within a j-tile
    for s, sl in enumerate(slots):
        qt, ct, jt = sl["qt"], sl["ct"], sl["jt"]
        for core, g in enumerate(sl["batches"]):
            oq = o_q[g]  # [Tq, H] f32
            oc = o_c[g]
            # oqT: [p, k*qt + j] = oq[j, k*128+p], + jt qbias columns
            oqT = np.empty((128, KT * qt + jt), np.float16)
            oqT[:, : KT * qt] = (
                oq[:qt].T.reshape(KT, 128, qt).transpose(1, 0, 2).reshape(128, KT * qt)
            )
            ql = int(q_lengths[g])
            tcol = np.arange(jt)[None, :] * 128 + jidx  # [128, jt]
            oqT[:, KT * qt :] = np.where(tcol < ql, np.float16(0.0), NEG16)
            # ocT: [p, k*ct + i] = oc[i, k*128+p]
            ocT = (
                oc[:ct].T.reshape(KT, 128, ct).transpose(1, 0, 2)
                .reshape(128, KT * ct)
            ).astype(np.float16)
            # oqN: per j-tile block [ones | oq rows]
            oqN = np.zeros((128, jt * OW), np.float16)
            for t in range(jt):
                oqN[:, t * OW] = 1.0
                oqN[:, t * OW + 1 : t * OW + 1 + H] = oq[t * 128 : (t + 1) * 128]
            maps[core][f"oqT{s}"] = np.ascontiguousarray(oqT)
            maps[core][f"ocT{s}"] = np.ascontiguousarray(ocT)
            maps[core][f"oqN{s}"] = np.ascontiguousarray(oqN)
    return maps


def kernel(**inputs) -> np.ndarray:
    o_c = np.asarray(inputs["o_c"], dtype=np.float32)
    o_q = np.asarray(inputs["o_q"], dtype=np.float32)
    W = np.asarray(inputs["W"], dtype=np.float32)
    q_lengths = np.asarray(inputs["q_lengths"]).astype(np.int64)
    c_lengths = np.asarray(inputs["c_lengths"]).astype(np.int64)
    # bias is mathematically irrelevant: it adds (o_c@b) per i-row before
    # softmax over j, which softmax cancels exactly.

    from concourse.bass_utils import run_bass_kernel_spmd

    slots = _plan(q_lengths, c_lengths)
    in_maps = _host_inputs(o_c, o_q, W, q_lengths, slots)
    nc = _build_program(slots)

    trace = bool(int(os.environ.get("KERNEL_TRACE", "0")))
    res = run_bass_kernel_spmd(
        nc, in_maps, core_ids=list(range(N_CORES)), trace=trace
    )
    if trace:
        kernel.last_results = res

    out = np.zeros((B, Tc, H), dtype=np.float32)
    for s, sl in enumerate(slots):
        for core, g in enumerate(sl["batches"]):
            cl = int(c_lengths[g])
            out[g, :cl] = res.results[core][f"out{s}"][:cl].astype(np.float32)
    return out



# revision 15
# speedup vs baseline: 1.1809x; 1.0025x over previous
"""C2Q attention kernel for 8 TRN2 NeuronCores — ragged-aware.

Math (per batch):
    score  = (o_c @ W @ o_q.T + (o_c @ b) 1^T) / sqrt(H)   [Tc, Tq]
    prob   = softmax_j(score masked at j>=q_len)
    out    = (prob * (i < c_len)) @ o_q                     [Tc, H]

Two exploits make the device program smaller than the dense math:
  * softmax is invariant to a per-row constant, so the bias term
    (o_c@b)1^T cancels exactly -> never computed.
  * by associativity the H x H projection can hit either side:
    (o_c @ W) @ o_q.T  or  o_c @ (W @ o_q.T).  Each slot picks the
    side with the shorter length, so the Linear costs 64*min(q,c)
    PE-rows instead of 64*Tq.

Ragged scheduling: the program is compiled AT RUNTIME for the actual
lengths.  The 32 batches are grouped into 4 slots x 8 cores so that
each slot's compile-time shape (q~, c~) = componentwise max over its 8
batches (grouping chosen by local search to minimize total PE rows).
All cores run the identical 4-slot program on their own batch of each
slot -> SPMD holds, but ~25% of the dense FLOPs are never issued.

Device layout per slot (everything lands K-on-partitions, no on-chip
transposes):
    proj   = 8 psum groups, free dim = min(q~,c~)     [128, 8*L] f16
    e[j,i] = exp(score/32 + qbias[j]) per j-tile      [<=128, c~] f16
             (qbias in {0,-60000} -> masked j gives exactly 0)
    ctx    = e.T @ [1 | o_q] in 3 free-blocks of ~342; the ones column
             makes d[i] = sum_j e[j,i] land in psum col 0, already
             per-partition -> reciprocal feeds the eviction scale.
c_len masking is host-side (only rows < c_len are copied out).
"""

import os
import sys

import numpy as np

if "/opt/trn_rl_repo" not in sys.path:
    sys.path.insert(0, "/opt/trn_rl_repo")

B, Tc, Tq, H = 32, 512, 512, 1024
N_CORES = 8
N_SLOTS = B // N_CORES  # 4
KT = H // 128  # contraction tiles over features (8)
OW = 1032  # oqN slab block width: [ones | h0..h1023] padded
SCALE = 1.0 / 32.0  # 1/sqrt(H)
NEG16 = np.float16(-60000.0)  # exp(x - 60000) == 0 exactly in fp32

CTX_BLOCKS = [(0, 342), (342, 684), (684, 1025)]  # cols of [1 | h...]


def _r16(x: int) -> int:
    return -(-int(x) // 16) * 16


def _rows(qm: int, cm: int) -> int:
    """PE row cost of one slot with shape (q~, c~)."""
    q, c = _r16(qm), _r16(cm)
    jt, it = -(-q // 128), -(-c // 128)
    return 64 * min(q, c) + 8 * jt * c + it * jt * 1025


def _group_batches(q_len, c_len):
    """Partition 32 batches into 4 groups of 8 minimizing slot-max cost."""
    import random

    rng = random.Random(12345)
    n = len(q_len)

    def total(groups):
        return sum(
            _rows(max(q_len[i] for i in g), max(c_len[i] for i in g))
            for g in groups
        )

    best_t, best_g = None, None
    for trial in range(12):
        order = sorted(
            range(n), key=lambda i: -(q_len[i] * 1024 + c_len[i])
        ) if trial == 0 else rng.sample(range(n), n)
        groups = [order[i * 8 : (i + 1) * 8] for i in range(N_SLOTS)]
        cur = total(groups)
        for _ in range(20000):
            g1, g2 = rng.sample(range(N_SLOTS), 2)
            i1, i2 = rng.randrange(8), rng.randrange(8)
            groups[g1][i1], groups[g2][i2] = groups[g2][i2], groups[g1][i1]
            t = total(groups)
            if t <= cur:
                cur = t
            else:
                groups[g1][i1], groups[g2][i2] = groups[g2][i2], groups[g1][i1]
        if best_t is None or cur < best_t:
            best_t, best_g = cur, [list(g) for g in groups]
    # order slots by descending cost: the big slot rides out the DMA ramp
    # (most PE work per input byte), the small slot gives a short tail
    costs = [
        _rows(max(q_len[i] for i in g), max(c_len[i] for i in g))
        for g in best_g
    ]
    order = sorted(range(N_SLOTS), key=lambda s: -costs[s])
    return [best_g[s] for s in order]


def _build_program(slots):
    """slots: list of dicts with qt, ct, jt, it, side ('q'|'c')."""
    import concourse.bacc as bacc
    import concourse.mybir as mybir
    import concourse.tile as tile

    f32 = mybir.dt.float32
    f16 = mybir.dt.float16
    nc = bacc.Bacc("TRN2", debug=False)

    need_q = any(s["side"] == "q" for s in slots)
    need_c = any(s["side"] == "c" for s in slots)

    wtq_d = nc.declare_dram_parameter("wtq", [128, 4, KT, 256], f16, isOutput=False) if need_q else None
    wtc_d = nc.declare_dram_parameter("wtc", [128, 4, KT, 256], f16, isOutput=False) if need_c else None
    oqT_d, ocT_d, oqN_d, out_d = [], [], [], []
    for s, sl in enumerate(slots):
        qt, ct, jt, it = sl["qt"], sl["ct"], sl["jt"], sl["it"]
        oqT_d.append(nc.declare_dram_parameter(f"oqT{s}", [128, KT * qt + jt], f16, isOutput=False))
        ocT_d.append(nc.declare_dram_parameter(f"ocT{s}", [128, KT * ct], f16, isOutput=False))
        oqN_d.append(nc.declare_dram_parameter(f"oqN{s}", [128, jt * OW], f16, isOutput=False))
        out_d.append(nc.declare_dram_parameter(f"out{s}", [ct, H], f16, isOutput=True))

    with tile.TileContext(nc) as tc:
        with (
            tc.tile_pool(name="const", bufs=1) as cpool,
            tc.tile_pool(name="inp", bufs=2) as ipool,
            tc.tile_pool(name="work", bufs=1) as wpool,
            tc.tile_pool(name="epool", bufs=2) as epool,
            tc.tile_pool(name="outp", bufs=3) as opool,
            tc.tile_pool(name="ps_acc", bufs=2, space="PSUM") as ps_acc,
            tc.tile_pool(name="ps_ctx", bufs=2, space="PSUM") as ps_ctx,
        ):
            wtq = cpool.tile([128, 4, KT, 256], f16, tag="wtq", name="wtq") if need_q else None
            wtc = cpool.tile([128, 4, KT, 256], f16, tag="wtc", name="wtc") if need_c else None

            # per-slot state carried between emission phases
            st = [dict() for _ in slots]

            def warmup():
                """~3us of throwaway matmuls at program start.  The PE's HAM
                clock gate needs ~3.4us of sustained activity to lift the
                default 4/8 throttle; these run during the input-DMA ramp
                (otherwise dead time) so the first real matmul starts at
                2.4GHz instead of 1.2."""
                wsrc = cpool.tile([128, 640], f16, tag="warm", name="warm_src")
                nc.vector.memset(wsrc[:, :], 0.0)
                for i in range(8):
                    wp = ps_acc.tile([128, 512], f32, tag="acc", name=f"warm{i}")
                    nc.tensor.matmul(
                        wp[:, :512], wsrc[:, :128], wsrc[:, 128:640],
                        start=True, stop=True,
                    )

            def dma_inputs(s, fine):
                sl = slots[s]
                qt, ct, jt = sl["qt"], sl["ct"], sl["jt"]
                oqT = ipool.tile([128, KT * qt + jt], f16, tag="oqT", name=f"oqT_{s}")
                ocT = ipool.tile([128, KT * ct], f16, tag="ocT", name=f"ocT_{s}")
                oqN = ipool.tile([128, jt * OW], f16, tag="oqN", name=f"oqN_{s}")
                st[s].update(oqT=oqT, ocT=ocT, oqN=oqN)
                if fine:
                    # ramp schedule: DMA bandwidth scales hard with packet
                    # size (~130GB/s at 1KB rows vs ~420GB/s at 4-8KB rows),
                    # so stream the moving tensor and the first wt o-block in
                    # k-HALF chunks: rows stay >=2KB contiguous while lin0's
                    # first og-pair can start after half 0 lands
                    w_slab = wtq if sl["side"] == "q" else wtc
                    w_d = wtq_d if sl["side"] == "q" else wtc_d
                    L = qt if sl["side"] == "q" else ct
                    mov, mov_d = (oqT, oqT_d[s]) if sl["side"] == "q" else (ocT, ocT_d[s])
                    oth, oth_d = (ocT, ocT_d[s]) if sl["side"] == "q" else (oqT, oqT_d[s])
                    # 4-way k-pair descriptors: a single DMA descriptor moves
                    # ~130-150GB/s regardless of size (engine-serial), so
                    # parallel descriptors on distinct engines are what buys
                    # aggregate bandwidth during the ramp
                    # issue order matters: the sync queue issues one
                    # descriptor per ~650ns, so wt block1 is hoisted between
                    # the k-chunks (og-pair 2 needs it ~9 matmul-groups in,
                    # and a tail-position issue left a ~2us PE gap)
                    kh = 2
                    for h in range(4):
                        lo, hi = h * kh * L, (h + 1) * kh * L
                        if h == 3 and mov is oqT:
                            hi += jt  # qb bias columns ride with the last chunk
                        nc.sync.dma_start(out=mov[:, lo:hi], in_=mov_d[:, lo:hi])
                        nc.sync.dma_start(
                            out=w_slab[:, 0:1, h * kh : (h + 1) * kh, :],
                            in_=w_d[:, 0:1, h * kh : (h + 1) * kh, :],
                        )
                        if h == 1:
                            nc.sync.dma_start(out=w_slab[:, 1], in_=w_d[:, 1])
                    nc.sync.dma_start(out=w_slab[:, 2], in_=w_d[:, 2])
                    # the score-side slab is needed ~12us in; for side-c it
                    # also carries the qb bias columns
                    nc.sync.dma_start(out=oth, in_=oth_d[:, :])
                    nc.sync.dma_start(out=w_slab[:, 3], in_=w_d[:, 3])
                    other_w = wtc if (sl["side"] == "q" and need_c) else (wtq if (sl["side"] == "c" and need_q) else None)
                    other_wd = wtc_d if sl["side"] == "q" else wtq_d
                    if other_w is not None:
                        nc.sync.dma_start(out=other_w[:, :2], in_=other_wd[:, :2])
                        nc.sync.dma_start(out=other_w[:, 2:], in_=other_wd[:, 2:])
                else:
                    nc.sync.dma_start(out=oqT, in_=oqT_d[s][:, :])
                    nc.sync.dma_start(out=ocT, in_=ocT_d[s][:, :])
                # all DMA stays on the sync queue: waking the GpSimd queue
                # costs the PE its boost p-state (measured 2.37 -> 2.0 GHz)
                nc.sync.dma_start(out=oqN, in_=oqN_d[s][:, :])

            def linear_gen(s):
                """Yield one emission step (matmul / eviction) at a time so
                ctx(s-1) can interleave them into its eviction bubbles."""
                sl = slots[s]
                qt, ct = sl["qt"], sl["ct"]
                L = qt if sl["side"] == "q" else ct
                w_slab = wtq if sl["side"] == "q" else wtc
                mov = st[s]["oqT"] if sl["side"] == "q" else st[s]["ocT"]
                proj = wpool.tile([128, KT * 512], f16, tag="proj", name=f"proj_{s}")
                st[s]["proj"] = proj
                for o in range(KT):
                    ups = ps_acc.tile([128, 512], f32, tag="acc", name=f"ups{o}_{s}")
                    for k in range(KT):
                        nc.tensor.matmul(
                            ups[:, :L],
                            w_slab[:, o // 2, k, (o % 2) * 128 : (o % 2 + 1) * 128],
                            mov[:, k * L : (k + 1) * L],
                            start=(k == 0),
                            stop=(k == KT - 1),
                        )
                        yield
                    nc.vector.tensor_scalar(
                        out=proj[:, o * L : (o + 1) * L],
                        in0=ups[:, :L],
                        scalar1=1.0,
                        scalar2=None,
                        op0=mybir.AluOpType.mult,
                    )

            def linear0():
                """Slot-0 Linear: o-groups 0/1 as a k-interleaved pair (each
                arriving oqT k-chunk feeds two matmuls, matching the DMA
                arrival rate during the ramp); o-groups 2-7 as single groups
                so the ps_acc ring-2 turnaround hides behind the previous
                group's 8 matmuls (pairs grab BOTH ring buffers at once and
                stall ~1us per pair on the eviction chain)."""
                sl = slots[0]
                qt, ct = sl["qt"], sl["ct"]
                L = qt if sl["side"] == "q" else ct
                w_slab = wtq if sl["side"] == "q" else wtc
                mov = st[0]["oqT"] if sl["side"] == "q" else st[0]["ocT"]
                proj = wpool.tile([128, KT * 512], f16, tag="proj", name="proj_0")
                st[0]["proj"] = proj

                def evict(o, up):
                    nc.vector.tensor_scalar(
                        out=proj[:, o * L : (o + 1) * L],
                        in0=up[:, :L],
                        scalar1=1.0,
                        scalar2=None,
                        op0=mybir.AluOpType.mult,
                    )

                ups = [
                    ps_acc.tile([128, 512], f32, tag="acc", name=f"ups{i}_0")
                    for i in range(2)
                ]
                for k in range(KT):
                    for i in range(2):
                        nc.tensor.matmul(
                            ups[i][:, :L],
                            w_slab[:, i // 2, k, (i % 2) * 128 : (i % 2 + 1) * 128],
                            mov[:, k * L : (k + 1) * L],
                            start=(k == 0),
                            stop=(k == KT - 1),
                        )
                        if k == KT - 1:
                            evict(i, ups[i])
                for o in range(2, KT):
                    up = ps_acc.tile([128, 512], f32, tag="acc", name=f"ups{o}_0")
                    for k in range(KT):
                        nc.tensor.matmul(
                            up[:, :L],
                            w_slab[:, o // 2, k, (o % 2) * 128 : (o % 2 + 1) * 128],
                            mov[:, k * L : (k + 1) * L],
                            start=(k == 0),
                            stop=(k == KT - 1),
                        )
                    evict(o, up)

            def drain(gen, n):
                if gen is None:
                    return
                for _ in range(n):
                    if next(gen, StopIteration) is StopIteration:
                        return

            def score(s):
                sl = slots[s]
                qt, ct, jt = sl["qt"], sl["ct"], sl["jt"]
                stat = st[s]["proj"] if sl["side"] == "q" else st[s]["oqT"]
                mov = st[s]["ocT"] if sl["side"] == "q" else st[s]["proj"]
                stat_L = qt  # j-slices always live in qt-wide sections
                mov_L = ct
                qb = st[s]["oqT"][:, KT * qt : KT * qt + jt]
                e_tiles = []
                for t in range(jt):
                    mj = min(128, qt - t * 128)
                    sps = ps_acc.tile([128, 512], f32, tag="acc", name=f"sps{t}_{s}")
                    for o in range(KT):
                        nc.tensor.matmul(
                            sps[:mj, :ct],
                            stat[:, o * stat_L + t * 128 : o * stat_L + t * 128 + mj],
                            mov[:, o * mov_L : (o + 1) * mov_L],
                            start=(o == 0),
                            stop=(o == KT - 1),
                        )
                    # ring-2: slot s+1's exp(t) can overwrite while ctx(s)
                    # still reads slot s's e tiles -> no PE bubble at the
                    # score(s+1) handoff
                    e = epool.tile([128, 512], f16, tag=f"e{t}", name=f"e{t}_{s}")
                    nc.scalar.activation(
                        out=e[:mj, :ct],
                        in_=sps[:mj, :ct],
                        func=mybir.ActivationFunctionType.Exp,
                        bias=qb[:mj, t : t + 1],
                        scale=SCALE,
                    )
                    e_tiles.append(e)
                st[s]["e"] = e_tiles

            def ctx(s, lin):
                """Emit ctx(s); weave next slot's Linear matmuls (lin gen)
                between psum groups so evictions never stall the PE."""
                sl = slots[s]
                qt, ct, jt, it = sl["qt"], sl["ct"], sl["jt"], sl["it"]
                e_tiles, oqN = st[s]["e"], st[s]["oqN"]
                drain(lin, 4)  # cover the last exp's latency
                last_tile = s == N_SLOTS - 1
                for ti in range(it):
                    mi = min(128, ct - ti * 128)
                    fin = last_tile and ti == it - 1
                    r = wpool.tile([128, 1], f32, tag="r", name=f"r{ti}_{s}")
                    osb = opool.tile([128, H], f16, tag="osb", name=f"osb{ti}_{s}")
                    cps = []
                    # evictions are pipelined INTO the block matmul stream:
                    # recip right after block0 (only needs psum col 0), b0/b1
                    # evictions while block2's matmuls stream, split ACT/DVE
                    # so neither engine serializes the i-tile tail.
                    for bi, (c0, c1) in enumerate(CTX_BLOCKS):
                        cp = ps_ctx.tile([128, 342], f32, tag=f"ctx{bi}", name=f"cps{ti}{bi}_{s}")
                        for t in range(jt):
                            mj = min(128, qt - t * 128)
                            nc.tensor.matmul(
                                cp[:mi, : c1 - c0],
                                e_tiles[t][:mj, ti * 128 : ti * 128 + mi],
                                oqN[:mj, t * OW + c0 : t * OW + c1],
                                start=(t == 0),
                                stop=(t == jt - 1),
                            )
                        cps.append(cp)
                        if bi == 0:
                            nc.vector.reciprocal(out=r[:mi], in_=cp[:mi, 0:1])
                        elif bi == 1:
                            nc.scalar.mul(
                                osb[:mi, 0:341], cps[0][:mi, 1:342], r[:mi]
                            )
                            nc.vector.tensor_scalar(
                                out=osb[:mi, 341:683],
                                in0=cps[1][:mi, 0:342],
                                scalar1=r[:mi],
                                scalar2=None,
                                op0=mybir.AluOpType.mult,
                            )
                        drain(lin, 6)
                    if fin:
                        # last eviction of the program: split ACT/DVE so the
                        # two halves run in parallel (~300ns vs ~650)
                        nc.scalar.mul(
                            osb[:mi, 683:853], cps[2][:mi, 0:170], r[:mi]
                        )
                        nc.vector.tensor_scalar(
                            out=osb[:mi, 853:1024],
                            in0=cps[2][:mi, 170:341],
                            scalar1=r[:mi],
                            scalar2=None,
                            op0=mybir.AluOpType.mult,
                        )
                    else:
                        nc.scalar.mul(
                            osb[:mi, 683:1024], cps[2][:mi, 0:341], r[:mi]
                        )
                    if fin:
                        # final transfer of the program: row-split into two
                        # full-width descriptors (2KB rows keep packets big;
                        # the transfers land on two DMA engines in parallel,
                        # halving the last transfer on the critical path)
                        h2 = mi // 2
                        nc.sync.dma_start(
                            out=out_d[s][ti * 128 : ti * 128 + h2, :],
                            in_=osb[:h2, :],
                        )
                        nc.sync.dma_start(
                            out=out_d[s][ti * 128 + h2 : ti * 128 + mi, :],
                            in_=osb[h2:mi, :],
                        )
                    else:
                        nc.sync.dma_start(
                            out=out_d[s][ti * 128 : ti * 128 + mi, :],
                            in_=osb[:mi, :],
                        )

            # PE order: warmup lin0 score0 | ctx0<<lin1 score1 | ctx1<<lin2
            #           score2 | ctx2<<lin3 score3 | ctx3
            warmup()
            dma_inputs(0, fine=True)
            dma_inputs(1, fine=False)
            linear0()
            score(0)
            for s in range(N_SLOTS):
                if s + 2 < N_SLOTS:
                    dma_inputs(s + 2, fine=False)
                lin = linear_gen(s + 1) if s + 1 < N_SLOTS else None
                ctx(s, lin)
                if lin is not None:
                    drain(lin, 1000)  # finish any remaining lin steps
                    score(s + 1)

    nc.compile()
    return nc


def _plan(q_lengths, c_lengths):
    groups = _group_batches(list(map(int, q_lengths)), list(map(int, c_lengths)))
    slots = []
    for g in groups:
        qt = _r16(max(int(q_lengths[i]) for i in g))
        ct = _r16(max(int(c_lengths[i]) for i in g))
        slots.append(
            dict(
                qt=qt, ct=ct,
                jt=-(-qt // 128), it=-(-ct // 128),
                side="q" if qt <= ct else "c",
                batches=list(g),
            )
        )
    return slots


def _host_inputs(o_c, o_q, W, q_lengths, slots):
    """Per-core input maps (host-side sharding + re-layout), all fp16."""
    need_q = any(s["side"] == "q" for s in slots)
    need_c = any(s["side"] == "c" for s in slots)
    maps = [dict() for _ in range(N_CORES)]
    if need_q:
        # wtq[p, ob, k, c] = W[ob*256 + c, k*128 + p]
        wtq = np.ascontiguousarray(
            W.reshape(4, 256, 8, 128).transpose(3, 0, 2, 1)
        ).astype(np.float16)
        for m in maps:
            m["wtq"] = wtq
    if need_c:
        # wtc[p, mb, k, c] = W[k*128 + p, mb*256 + c]
        wtc = np.ascontiguousarray(
            W.reshape(8, 128, 4, 256).transpose(1, 2, 0, 3)
        ).astype(np.float16)
        for m in maps:
            m["wtc"] = wtc

    jidx = np.arange(128)[:, None]  # partition index within a j-tile
    for s, sl in enumerate(slots):
        qt, ct, jt = sl["qt"], sl["ct"], sl["jt"]
        for core, g in enumerate(sl["batches"]):
            oq = o_q[g]  # [Tq, H] f32
            oc = o_c[g]
            # oqT: [p, k*qt + j] = oq[j, k*128+p], + jt qbias columns
            oqT = np.empty((128, KT * qt + jt), np.float16)
            oqT[:, : KT * qt] = (
                oq[:qt].T.reshape(KT, 128, qt).transpose(1, 0, 2).reshape(128, KT * qt)
            )
            ql = int(q_lengths[g])
            tcol = np.arange(jt)[None, :] * 128 + jidx  # [128, jt]
            oqT[:, KT * qt :] = np.where(tcol < ql, np.float16(0.0), NEG16)
            # ocT: [p, k*ct + i] = oc[i, k*128+p]
            ocT = (
                oc[:ct].T.reshape(KT, 128, ct).transpose(1, 0, 2)
                .reshape(128, KT * ct)
            ).astype(np.float16)
            # oqN: per j-tile block [ones | oq rows]
            oqN = np.zeros((128, jt * OW), np.float16)
            for t in range(jt):
                oqN[:, t * OW] = 1.0
                oqN[:, t * OW + 1 : t * OW + 1 + H] = oq[t * 128 : (t + 1) * 128]
            maps[core][f"oqT{s}"] = np.ascontiguousarray(oqT)
            maps[core][f"ocT{s}"] = np.ascontiguousarray(ocT)
            maps[core][f"oqN{s}"] = np.ascontiguousarray(oqN)
    return maps


def kernel(**inputs) -> np.ndarray:
    o_c = np.asarray(inputs["o_c"], dtype=np.float32)
    o_q = np.asarray(inputs["o_q"], dtype=np.float32)
    W = np.asarray(inputs["W"], dtype=np.float32)
    q_lengths = np.asarray(inputs["q_lengths"]).astype(np.int64)
    c_lengths = np.asarray(inputs["c_lengths"]).astype(np.int64)
    # bias is mathematically irrelevant: it adds (o_c@b) per i-row before
    # softmax over j, which softmax cancels exactly.

    from concourse.bass_utils import run_bass_kernel_spmd

    slots = _plan(q_lengths, c_lengths)
    in_maps = _host_inputs(o_c, o_q, W, q_lengths, slots)
    nc = _build_program(slots)

    trace = bool(int(os.environ.get("KERNEL_TRACE", "0")))
    res = run_bass_kernel_spmd(
        nc, in_maps, core_ids=list(range(N_CORES)), trace=trace
    )
    if trace:
        kernel.last_results = res

    out = np.zeros((B, Tc, H), dtype=np.float32)
    for s, sl in enumerate(slots):
        for core, g in enumerate(sl["batches"]):
            cl = int(c_lengths[g])
            out[g, :cl] = res.results[core][f"out{s}"][:cl].astype(np.float32)
    return out

